# revision 11
# baseline (speedup 1.0000x reference)
"""DeepSet segment-reduce kernel for 8 Trainium2 NeuronCores (Bass/Tile).

Math (reference):
    h  = relu(x1 @ W1 + b1) @ W2 + b2          # [E, 128]
    S  = segment_sum(h, seg)                   # [B, 128]
    mean = S / max(counts, 1)
    out  = mean @ W3 + b3

segment_sum is linear, so only r = relu(x1 @ W1 + b1) needs per-edge work:
segsum(h) = segsum(r) @ W2 + counts x b2.  The device computes per-PIECE
sums of r with FUSED relu+accumulate instructions; everything downstream
(piece->segment combine, the tiny W2/W3 stage) runs on the host (0.2% of
the FLOPs).

Device pipeline per core (one SPMD program on 8 cores):
  - two input regions: x columns of LARGE segments (count >= T_SMALL) in
    fp8 e4m3 scaled by 8 (quantization noise averages out over the
    segment mean), small segments in bf16
  - hT = W1.T @ xT on PE (bf16 stationary W1; fp8/bf16 moving) -> PSUM
  - per piece (a run of columns belonging to one segment): ONE fused
    instruction computing relu(h+b1) -> scratch and sum -> accum slot:
      ACT:  activation(Relu, bias=b1, accum_out=slot)
      DVE:  scalar_tensor_tensor((h+b1) max zeros, accum_out=slot)
    pieces are split between ACT and DVE by a cost model so both engines
    finish together.
  - accum [128, S] fp32 DMA'd out at the end.

SPMD uniformity: piece lengths are baked into the instruction stream, so
all 8 cores share one schedule per region. The host pads each segment to
a multiple of 128 cols (zero columns; their relu(b1) contribution is
subtracted on the host), chops segments into pieces <= 2048 (PSUM tile),
then splits pieces until each piece-length count is divisible by 8, so
every core gets an identical multiset of piece lengths. Any piece may go
to any core; the host epilogue recombines by segment id.

Self-contained: no reads of /root/problem/*; shapes derived from inputs.
"""

import numpy as np

N_CORES = 8
BLOCK = 128          # segment padding quantum (cols)
PIECE_MAX = 2048     # max piece length == PSUM tile cols (4 banks fp32)
PSUM_TILE = 2048
DMA_BYTES = 16384    # bytes per partition row per DMA (2 MiB transfers)
T_SMALL = 512        # segments below this edge count stay bf16
SCALE8 = 8.0         # fp8 region input scale (pow2; relu-commutes)

# per-piece engine cost model (ns), for ACT/DVE load balancing
ACT_FIX, ACT_PER = 430.0, 1.0 / 1.2
DVE_FIX, DVE_PER = 210.0, 1.0 / 0.96


def _dtypes():
    import ml_dtypes
    return ml_dtypes.bfloat16, ml_dtypes.float8_e4m3


def _plan_class(seg_ids, counts, dma_cols):
    """Piece schedule for one dtype class. dma_cols = cols per DMA tile."""
    per_dma = dma_cols // PSUM_TILE
    by_len = {L: [] for L in range(BLOCK, PIECE_MAX + 1, BLOCK)}
    for b in seg_ids:
        c = int(counts[b])
        if c == 0:
            continue
        total = ((c + BLOCK - 1) // BLOCK) * BLOCK
        start = 0
        while total > 0:
            L = min(total, PIECE_MAX)
            by_len[L].append((b, start, L))
            start += L
            total -= L

    # make every length-count divisible by N_CORES by splitting pieces
    for L in range(PIECE_MAX, BLOCK, -BLOCK):
        lst = by_len[L]
        for _ in range(len(lst) % N_CORES):
            seg, st, _ = lst.pop()
            nb = L // BLOCK
            L1 = (nb - nb // 2) * BLOCK
            L2 = L - L1
            by_len[L1].append((seg, st, L1))
            by_len[L2].append((seg, st + L1, L2))
    for _ in range((-len(by_len[BLOCK])) % N_CORES):
        by_len[BLOCK].append((-1, 0, BLOCK))  # dummy (all-zero cols)

    per_core_k = {L: len(by_len[L]) // N_CORES for L in by_len}

    # bin-pack the per-core multiset into PSUM tiles (first-fit decreasing)
    items = []
    for L in range(PIECE_MAX, BLOCK - 1, -BLOCK):
        items += [L] * per_core_k[L]
    bins = []
    for L in items:
        for b_ in bins:
            if b_[1] >= L:
                b_[0].append(L)
                b_[1] -= L
                break
        else:
            bins.append([[L], PIECE_MAX - L])
    n_bins = len(bins)
    while n_bins % per_dma:
        bins.append([[], PIECE_MAX])
        n_bins += 1
    e_cap = n_bins * PSUM_TILE

    # schedule entries + greedy ACT/DVE balance (largest first)
    entries = []
    for t, (lens, _) in enumerate(bins):
        off = 0
        for L in lens:
            entries.append([t, off, L])
            off += L
    order = sorted(range(len(entries)), key=lambda i: -entries[i][2])
    eng = [0] * len(entries)
    tA = tD = 0.0
    for i in order:
        fd = entries[i][2]
        cA = ACT_FIX + ACT_PER * fd
        cD = DVE_FIX + DVE_PER * fd
        if tA + cA <= tD + cD:
            eng[i] = 0
            tA += cA
        else:
            eng[i] = 1
            tD += cD
    sched = [(t, off, fd, eng[i], i)
             for i, (t, off, fd) in enumerate(entries)]
    n_slots = len(sched)

    slots_by_len = {}
    for (t, off, fd, e_, slot) in sched:
        slots_by_len.setdefault(fd, []).append(slot)
    core_pieces = []
    for c in range(N_CORES):
        pieces = [None] * n_slots
        for L, slots in slots_by_len.items():
            k = per_core_k[L]
            mine = by_len[L][c * k:(c + 1) * k]
            for s_, p_ in zip(slots, mine):
                if p_[0] >= 0:
                    pieces[s_] = p_
        core_pieces.append(pieces)

    return {"e_cap": e_cap, "sched": sched, "n_slots": n_slots,
            "core_pieces": core_pieces, "dma_cols": dma_cols}


def _plan(edge_slices, E, B):
    es = np.asarray(edge_slices, dtype=np.int64)
    counts = (es[1:] - es[:-1]).astype(np.int64)
    big = [b for b in range(B) if counts[b] >= T_SMALL]
    small = [b for b in range(B) if 0 < counts[b] < T_SMALL]
    p8 = _plan_class(big, counts, DMA_BYTES)        # fp8: 1 B/col
    p16 = _plan_class(small, counts, DMA_BYTES // 2)  # bf16: 2 B/col
    return {"es": es, "counts": counts, "B": B, "p8": p8, "p16": p16}


def _fill_region(xT, es, plan_c, dtype, e_cap):
    xs, npads = [], []
    slot_pos = {s: (t * PSUM_TILE + off, fd)
                for (t, off, fd, e_, s) in plan_c["sched"]}
    for c in range(N_CORES):
        xc = np.zeros((128, e_cap), dtype=dtype)
        npad = np.zeros(plan_c["n_slots"], dtype=np.int64)
        for s, piece in enumerate(plan_c["core_pieces"][c]):
            if piece is None:
                continue
            col0, fd = slot_pos[s]
            seg, st, L = piece
            a = es[seg] + st
            real = min(L, int(es[seg + 1] - a))
            if real > 0:
                xc[:, col0:col0 + real] = xT[:, a:a + real]
            npad[s] = L - max(real, 0)
        xs.append(xc)
        npads.append(npad)
    return xs, npads


def _build_core_inputs(x1, plan):
    bf16, f8 = _dtypes()
    es = plan["es"]
    xT = np.ascontiguousarray(x1.T)  # [128, E] f32
    xT8 = (xT * SCALE8).astype(f8)
    xT16 = xT.astype(bf16)
    xs8, npads8 = _fill_region(xT8, es, plan["p8"], f8, plan["p8"]["e_cap"])
    xs16, npads16 = _fill_region(xT16, es, plan["p16"], bf16,
                                 plan["p16"]["e_cap"])
    plan["npads8"] = npads8
    plan["npads16"] = npads16
    return xs8, xs16


def _build_bass(p8, p16):
    import concourse.bacc as bacc
    import concourse.mybir as mybir
    import concourse.tile as tile

    f32 = mybir.dt.float32
    bf = mybir.dt.bfloat16
    f8 = mybir.dt.float8e4
    Relu = mybir.ActivationFunctionType.Relu
    Max = mybir.AluOpType.max
    Add = mybir.AluOpType.add

    nc = bacc.Bacc(trn_type="TRN2", num_devices=N_CORES)

    x8_d = nc.dram_tensor("xT8", [128, p8["e_cap"]], f8, kind="ExternalInput")
    x16_d = nc.dram_tensor("xT16", [128, p16["e_cap"]], bf,
                           kind="ExternalInput")
    W1_d = nc.dram_tensor("W1b", [128, 128], bf, kind="ExternalInput")
    b18_d = nc.dram_tensor("b1c8", [128, 1], f32, kind="ExternalInput")
    b116_d = nc.dram_tensor("b1c16", [128, 1], f32, kind="ExternalInput")
    n_slots = p8["n_slots"] + p16["n_slots"]
    acc_d = nc.dram_tensor("acc", [128, n_slots], f32, kind="ExternalOutput")

    with tile.TileContext(nc) as tc, tc.tile_pool(name="persist", bufs=1) as pp:
        w1_sb = pp.tile([128, 128], bf, name="w1_sb")
        b18_sb = pp.tile([128, 1], f32, name="b18_sb")
        b116_sb = pp.tile([128, 1], f32, name="b116_sb")
        zero_sb = pp.tile([128, PSUM_TILE], f32, name="zero_sb")
        acc_sb = pp.tile([128, n_slots], f32, name="acc_sb")
        nc.sync.dma_start(w1_sb[:], W1_d[:])
        nc.sync.dma_start(b18_sb[:], b18_d[:])
        nc.sync.dma_start(b116_sb[:], b116_d[:])
        nc.vector.memset(zero_sb[:], 0.0)

        with (
            tc.tile_pool(name="xp8", bufs=3) as xp8,
            tc.tile_pool(name="xp16", bufs=3) as xp16,
            tc.tile_pool(name="hp", bufs=2, space="PSUM") as hp,
            tc.tile_pool(name="sa", bufs=2) as sa,
            tc.tile_pool(name="sd", bufs=2) as sd,
        ):
            warmed = [False]

            def emit_region(plan_c, x_d, xdt, xp, b1_ap, slot_base):
                dma_cols = plan_c["dma_cols"]
                per_dma = dma_cols // PSUM_TILE
                n_dma = plan_c["e_cap"] // dma_cols
                sched_by_tile = {}
                for (t, off, fd, e_, slot) in plan_c["sched"]:
                    sched_by_tile.setdefault(t, []).append(
                        (off, fd, e_, slot + slot_base))
                for t in range(n_dma):
                    xt = xp.tile([128, dma_cols], xdt, name="xt")
                    nc.sync.dma_start(
                        xt[:], x_d[:, t * dma_cols:(t + 1) * dma_cols])
                    for h in range(per_dma):
                        ps = hp.tile([128, PSUM_TILE], f32, name="ps")
                        if not warmed[0]:
                            # HAM warm-up: ~4.5us of back-to-back matmuls
                            # so the PE clock gate opens (K=8/8); the
                            # first real matmul's start=True overwrites.
                            for _ in range(42):
                                nc.tensor.matmul(
                                    ps[:, 0:128], lhsT=w1_sb[:],
                                    rhs=w1_sb[:], start=True, stop=True)
                            warmed[0] = True
                        for q in range(PSUM_TILE // 512):
                            c0 = h * PSUM_TILE + q * 512
                            sl = slice(q * 512, (q + 1) * 512)
                            nc.tensor.matmul(
                                ps[:, sl], lhsT=w1_sb[:],
                                rhs=xt[:, c0:c0 + 512],
                                start=True, stop=True)
                        for (off, fd, e_, slot) in sched_by_tile.get(t * per_dma + h, []):
                            acc_ap = acc_sb[:, slot:slot + 1]
                            if e_ == 0:
                                sc = sa.tile([128, PSUM_TILE], bf, name="sca")
                                nc.scalar.activation(
                                    sc[:, :fd], ps[:, off:off + fd], Relu,
                                    bias=b1_ap, accum_out=acc_ap)
                            else:
                                sc = sd.tile([128, PSUM_TILE], bf, name="scd")
                                nc.vector.scalar_tensor_tensor(
                                    sc[:, :fd], ps[:, off:off + fd],
                                    b1_ap, zero_sb[:, :fd],
                                    op0=Add, op1=Max, accum_out=acc_ap)

            emit_region(p8, x8_d, f8, xp8, b18_sb[:, 0:1], 0)
            emit_region(p16, x16_d, bf, xp16, b116_sb[:, 0:1], p8["n_slots"])

        nc.sync.dma_start(acc_d[:], acc_sb[:])

    nc.compile()
    return nc


def _prepare(x1, edge_slices, W1, b1, W2, b2, W3, b3):
    bf16, _ = _dtypes()
    x1 = np.ascontiguousarray(np.asarray(x1, dtype=np.float32))
    E = x1.shape[0]
    B = int(np.asarray(edge_slices).shape[0]) - 1

    plan = _plan(edge_slices, E, B)
    xs8, xs16 = _build_core_inputs(x1, plan)

    b1f = np.asarray(b1, np.float32).reshape(128, 1)
    shared = {
        "W1b": np.ascontiguousarray(np.asarray(W1, np.float32).astype(bf16)),
        "b1c8": np.ascontiguousarray(b1f * SCALE8),
        "b1c16": np.ascontiguousarray(b1f),
    }
    nc = _build_bass(plan["p8"], plan["p16"])
    in_maps = [{"xT8": xs8[c], "xT16": xs16[c], **shared}
               for c in range(N_CORES)]
    return nc, in_maps, plan


def _finish(acc_list, plan, b1, W2, b2, W3, b3):
    """Host epilogue: piece sums -> segment sums -> mean -> W3."""
    B = plan["B"]
    counts = plan["counts"].astype(np.float32)
    # zero-pad cols produce relu(0 @ W1 + b1*s) = s*relu(b1): subtract
    relu_b1 = np.maximum(np.asarray(b1, np.float32), 0.0)

    S8 = plan["p8"]["n_slots"]
    R = np.zeros((B, 128), dtype=np.float64)
    for c in range(N_CORES):
        acc = np.asarray(acc_list[c], np.float64)
        for plan_c, npads, base, scale in (
                (plan["p8"], plan["npads8"][c], 0, SCALE8),
                (plan["p16"], plan["npads16"][c], S8, 1.0)):
            for s, piece in enumerate(plan_c["core_pieces"][c]):
                if piece is None:
                    continue
                seg = piece[0]
                R[seg] += acc[:, base + s] / scale
                if npads[s]:
                    R[seg] -= npads[s] * relu_b1
    R = R.astype(np.float32)

    W2 = np.asarray(W2, np.float32)
    b2 = np.asarray(b2, np.float32)
    W3 = np.asarray(W3, np.float32)
    b3 = np.asarray(b3, np.float32)
    sums_h = R @ W2 + counts[:, None] * b2[None, :]
    mean = sums_h / np.maximum(counts, 1.0)[:, None]
    return (mean @ W3 + b3[None, :]).astype(np.float32)


def kernel(x1, edge_slices, W1, b1, W2, b2, W3, b3):
    from concourse import bass_utils

    nc, in_maps, plan = _prepare(x1, edge_slices, W1, b1, W2, b2, W3, b3)
    br = bass_utils.run_bass_kernel_spmd(
        nc, in_maps, core_ids=list(range(N_CORES)))
    return _finish([r["acc"] for r in br.results], plan, b1, W2, b2, W3, b3)


# revision 18
# speedup vs baseline: 1.1982x; 1.1982x over previous
"""DeepSet segment-reduce kernel for 8 Trainium2 NeuronCores (Bass/Tile).

Math (reference):
    h  = relu(x1 @ W1 + b1) @ W2 + b2          # [E, 128]
    S  = segment_sum(h, seg)                   # [B, 128]
    mean = S / max(counts, 1)
    out  = mean @ W3 + b3

segment_sum is linear, so only r = relu(x1 @ W1 + b1) needs per-edge work:
segsum(h) = segsum(r) @ W2 + counts x b2.  The device computes per-PIECE
sums of r with FUSED relu+accumulate instructions; everything downstream
(piece->segment combine, the tiny W2/W3 stage) runs on the host (0.2% of
the FLOPs).

Device pipeline per core (one SPMD program on 8 cores):
  - two input regions: x columns of LARGE segments (count >= T_SMALL) in
    fp8 e4m3 scaled by 8 (quantization noise averages out over the
    segment mean), small segments in bf16
  - hT = W1.T @ xT on PE (bf16 stationary W1; fp8/bf16 moving) -> PSUM
  - per piece (a run of columns belonging to one segment): ONE fused
    instruction computing relu(h+b1) -> scratch and sum -> accum slot:
      ACT:  activation(Relu, bias=b1, accum_out=slot)
      DVE:  scalar_tensor_tensor((h+b1) max zeros, accum_out=slot)
    pieces are split between ACT and DVE by a cost model so both engines
    finish together.
  - accum [128, S] fp32 DMA'd out at the end.

SPMD uniformity: piece lengths are baked into the instruction stream, so
all 8 cores share one schedule per region. The host pads each segment to
a multiple of 128 cols (zero columns; their relu(b1) contribution is
subtracted on the host), chops segments into pieces <= 2048 (PSUM tile),
then splits pieces until each piece-length count is divisible by 8, so
every core gets an identical multiset of piece lengths. Any piece may go
to any core; the host epilogue recombines by segment id.

Self-contained: no reads of /root/problem/*; shapes derived from inputs.
"""

import numpy as np

N_CORES = 8
BLOCK = 128          # segment padding quantum (cols)
PIECE_MAX = 2048     # max piece length == PSUM tile cols (4 banks fp32)
PSUM_TILE = 2048
DMA_BYTES = 16384    # bytes per partition row per DMA (2 MiB transfers)
T_SMALL = 512        # segments below this edge count stay bf16
SCALE8 = 8.0         # fp8 region input scale (pow2; relu-commutes)

# per-piece engine cost model (ns), for ACT/DVE load balancing
ACT_FIX, ACT_PER = 430.0, 1.0 / 1.2
DVE_FIX, DVE_PER = 210.0, 1.0 / 0.96


def _dtypes():
    import ml_dtypes
    return ml_dtypes.bfloat16, ml_dtypes.float8_e4m3


def _plan_class(seg_ids, counts, dma_cols):
    """Piece schedule for one dtype class. dma_cols = cols per DMA tile."""
    per_dma = dma_cols // PSUM_TILE
    by_len = {L: [] for L in range(BLOCK, PIECE_MAX + 1, BLOCK)}
    for b in seg_ids:
        c = int(counts[b])
        if c == 0:
            continue
        total = ((c + BLOCK - 1) // BLOCK) * BLOCK
        start = 0
        while total > 0:
            L = min(total, PIECE_MAX)
            by_len[L].append((b, start, L))
            start += L
            total -= L

    # make every length-count divisible by N_CORES by splitting pieces
    for L in range(PIECE_MAX, BLOCK, -BLOCK):
        lst = by_len[L]
        for _ in range(len(lst) % N_CORES):
            seg, st, _ = lst.pop()
            nb = L // BLOCK
            L1 = (nb - nb // 2) * BLOCK
            L2 = L - L1
            by_len[L1].append((seg, st, L1))
            by_len[L2].append((seg, st + L1, L2))
    for _ in range((-len(by_len[BLOCK])) % N_CORES):
        by_len[BLOCK].append((-1, 0, BLOCK))  # dummy (all-zero cols)

    per_core_k = {L: len(by_len[L]) // N_CORES for L in by_len}

    # bin-pack the per-core multiset into PSUM tiles (first-fit decreasing)
    items = []
    for L in range(PIECE_MAX, BLOCK - 1, -BLOCK):
        items += [L] * per_core_k[L]
    bins = []
    for L in items:
        for b_ in bins:
            if b_[1] >= L:
                b_[0].append(L)
                b_[1] -= L
                break
        else:
            bins.append([[L], PIECE_MAX - L])
    n_bins = len(bins)
    while n_bins % per_dma:
        bins.append([[], PIECE_MAX])
        n_bins += 1
    e_cap = n_bins * PSUM_TILE

    # schedule entries + greedy ACT/DVE balance (largest first)
    entries = []
    for t, (lens, _) in enumerate(bins):
        off = 0
        for L in lens:
            entries.append([t, off, L])
            off += L
    order = sorted(range(len(entries)), key=lambda i: -entries[i][2])
    eng = [0] * len(entries)
    tA = tD = 0.0
    for i in order:
        fd = entries[i][2]
        cA = ACT_FIX + ACT_PER * fd
        cD = DVE_FIX + DVE_PER * fd
        if tA + cA <= tD + cD:
            eng[i] = 0
            tA += cA
        else:
            eng[i] = 1
            tD += cD
    sched = [(t, off, fd, eng[i], i)
             for i, (t, off, fd) in enumerate(entries)]
    n_slots = len(sched)

    # per-engine accumulator column (ACT and DVE write separate SBUF
    # tiles so the accum streams run concurrently - a shared tile
    # serializes the engines via WAW tracking)
    col_of = [0] * n_slots
    nA = nD = 0
    for (t, off, fd, e_, slot) in sched:
        if e_ == 0:
            col_of[slot] = nA
            nA += 1
        else:
            col_of[slot] = nD
            nD += 1

    slots_by_len = {}
    for (t, off, fd, e_, slot) in sched:
        slots_by_len.setdefault(fd, []).append(slot)
    core_pieces = []
    for c in range(N_CORES):
        pieces = [None] * n_slots
        for L, slots in slots_by_len.items():
            k = per_core_k[L]
            mine = by_len[L][c * k:(c + 1) * k]
            for s_, p_ in zip(slots, mine):
                if p_[0] >= 0:
                    pieces[s_] = p_
        core_pieces.append(pieces)

    return {"e_cap": e_cap, "sched": sched, "n_slots": n_slots,
            "core_pieces": core_pieces, "dma_cols": dma_cols,
            "col_of": col_of, "nA": nA, "nD": nD}


def _plan(edge_slices, E, B):
    es = np.asarray(edge_slices, dtype=np.int64)
    counts = (es[1:] - es[:-1]).astype(np.int64)
    big = [b for b in range(B) if counts[b] >= T_SMALL]
    small = [b for b in range(B) if 0 < counts[b] < T_SMALL]
    p8 = _plan_class(big, counts, DMA_BYTES)        # fp8: 1 B/col
    p16 = _plan_class(small, counts, DMA_BYTES // 2)  # bf16: 2 B/col
    return {"es": es, "counts": counts, "B": B, "p8": p8, "p16": p16}


def _fill_region(xT, es, plan_c, dtype, e_cap):
    xs, npads = [], []
    slot_pos = {s: (t * PSUM_TILE + off, fd)
                for (t, off, fd, e_, s) in plan_c["sched"]}
    for c in range(N_CORES):
        xc = np.zeros((128, e_cap), dtype=dtype)
        npad = np.zeros(plan_c["n_slots"], dtype=np.int64)
        for s, piece in enumerate(plan_c["core_pieces"][c]):
            if piece is None:
                continue
            col0, fd = slot_pos[s]
            seg, st, L = piece
            a = es[seg] + st
            real = min(L, int(es[seg + 1] - a))
            if real > 0:
                xc[:, col0:col0 + real] = xT[:, a:a + real]
            npad[s] = L - max(real, 0)
        xs.append(xc)
        npads.append(npad)
    return xs, npads


def _build_core_inputs(x1, plan):
    bf16, f8 = _dtypes()
    es = plan["es"]
    xT = np.ascontiguousarray(x1.T)  # [128, E] f32
    xT8 = (xT * SCALE8).astype(f8)
    xT16 = xT.astype(bf16)
    xs8, npads8 = _fill_region(xT8, es, plan["p8"], f8, plan["p8"]["e_cap"])
    xs16, npads16 = _fill_region(xT16, es, plan["p16"], bf16,
                                 plan["p16"]["e_cap"])
    plan["npads8"] = npads8
    plan["npads16"] = npads16
    return xs8, xs16


def _build_bass(p8, p16):
    import concourse.bacc as bacc
    import concourse.mybir as mybir
    import concourse.tile as tile

    f32 = mybir.dt.float32
    bf = mybir.dt.bfloat16
    f8 = mybir.dt.float8e4
    Relu = mybir.ActivationFunctionType.Relu
    Max = mybir.AluOpType.max
    Add = mybir.AluOpType.add

    nc = bacc.Bacc(trn_type="TRN2", num_devices=N_CORES)

    x8_d = nc.dram_tensor("xT8", [128, p8["e_cap"]], f8, kind="ExternalInput")
    x16_d = nc.dram_tensor("xT16", [128, p16["e_cap"]], bf,
                           kind="ExternalInput")
    W1_d = nc.dram_tensor("W1b", [128, 128], bf, kind="ExternalInput")
    b18_d = nc.dram_tensor("b1c8", [128, 1], f32, kind="ExternalInput")
    b116_d = nc.dram_tensor("b1c16", [128, 1], f32, kind="ExternalInput")
    nA = p8["nA"] + p16["nA"]
    nD = p8["nD"] + p16["nD"]
    acc_d = nc.dram_tensor("acc", [128, nA + nD], f32, kind="ExternalOutput")

    with tile.TileContext(nc) as tc, tc.tile_pool(name="persist", bufs=1) as pp:
        w1_sb = pp.tile([128, 128], bf, name="w1_sb")
        b18_sb = pp.tile([128, 1], f32, name="b18_sb")
        b116_sb = pp.tile([128, 1], f32, name="b116_sb")
        zero_sb = pp.tile([128, PSUM_TILE], f32, name="zero_sb")
        accA_sb = pp.tile([128, max(nA, 1)], f32, name="accA_sb")
        accD_sb = pp.tile([128, max(nD, 1)], f32, name="accD_sb")
        nc.sync.dma_start(w1_sb[:], W1_d[:])
        nc.sync.dma_start(b18_sb[:], b18_d[:])
        nc.sync.dma_start(b116_sb[:], b116_d[:])
        nc.vector.memset(zero_sb[:], 0.0)

        with (
            tc.tile_pool(name="xp8", bufs=3) as xp8,
            tc.tile_pool(name="xp16", bufs=3) as xp16,
            tc.tile_pool(name="hp", bufs=2, space="PSUM") as hp,
            tc.tile_pool(name="sa", bufs=2) as sa,
            tc.tile_pool(name="sd", bufs=2) as sd,
        ):
            warmed = [False]

            def emit_region(plan_c, x_d, xdt, xp, b1_ap, baseA, baseD):
                dma_cols = plan_c["dma_cols"]
                per_dma = dma_cols // PSUM_TILE
                n_dma = plan_c["e_cap"] // dma_cols
                sched_by_tile = {}
                for (t, off, fd, e_, slot) in plan_c["sched"]:
                    col = plan_c["col_of"][slot] + (baseA if e_ == 0 else baseD)
                    sched_by_tile.setdefault(t, []).append((off, fd, e_, col))
                for t in range(n_dma):
                    xt = xp.tile([128, dma_cols], xdt, name="xt")
                    nc.sync.dma_start(
                        xt[:], x_d[:, t * dma_cols:(t + 1) * dma_cols])
                    for h in range(per_dma):
                        ps = hp.tile([128, PSUM_TILE], f32, name="ps")
                        if not warmed[0]:
                            # HAM warm-up: ~4.5us of back-to-back matmuls
                            # so the PE clock gate opens (K=8/8); the
                            # first real matmul's start=True overwrites.
                            for _ in range(42):
                                nc.tensor.matmul(
                                    ps[:, 0:128], lhsT=w1_sb[:],
                                    rhs=w1_sb[:], start=True, stop=True)
                            warmed[0] = True
                        for q in range(PSUM_TILE // 512):
                            c0 = h * PSUM_TILE + q * 512
                            sl = slice(q * 512, (q + 1) * 512)
                            nc.tensor.matmul(
                                ps[:, sl], lhsT=w1_sb[:],
                                rhs=xt[:, c0:c0 + 512],
                                start=True, stop=True)
                        for (off, fd, e_, col) in sched_by_tile.get(t * per_dma + h, []):
                            acc_ap = (accA_sb[:, col:col + 1] if e_ == 0
                                      else accD_sb[:, col:col + 1])
                            if e_ == 0:
                                sc = sa.tile([128, PSUM_TILE], bf, name="sca")
                                nc.scalar.activation(
                                    sc[:, :fd], ps[:, off:off + fd], Relu,
                                    bias=b1_ap, accum_out=acc_ap)
                            else:
                                sc = sd.tile([128, PSUM_TILE], bf, name="scd")
                                nc.vector.scalar_tensor_tensor(
                                    sc[:, :fd], ps[:, off:off + fd],
                                    b1_ap, zero_sb[:, :fd],
                                    op0=Add, op1=Max, accum_out=acc_ap)

            emit_region(p8, x8_d, f8, xp8, b18_sb[:, 0:1], 0, 0)
            emit_region(p16, x16_d, bf, xp16, b116_sb[:, 0:1],
                        p8["nA"], p8["nD"])

        if nA:
            nc.sync.dma_start(acc_d[:, 0:nA], accA_sb[:])
        if nD:
            nc.sync.dma_start(acc_d[:, nA:nA + nD], accD_sb[:])

    nc.compile()
    return nc


def _prepare(x1, edge_slices, W1, b1, W2, b2, W3, b3):
    bf16, _ = _dtypes()
    x1 = np.ascontiguousarray(np.asarray(x1, dtype=np.float32))
    E = x1.shape[0]
    B = int(np.asarray(edge_slices).shape[0]) - 1

    plan = _plan(edge_slices, E, B)
    xs8, xs16 = _build_core_inputs(x1, plan)

    b1f = np.asarray(b1, np.float32).reshape(128, 1)
    shared = {
        "W1b": np.ascontiguousarray(np.asarray(W1, np.float32).astype(bf16)),
        "b1c8": np.ascontiguousarray(b1f * SCALE8),
        "b1c16": np.ascontiguousarray(b1f),
    }
    nc = _build_bass(plan["p8"], plan["p16"])
    in_maps = [{"xT8": xs8[c], "xT16": xs16[c], **shared}
               for c in range(N_CORES)]
    return nc, in_maps, plan


def _finish(acc_list, plan, b1, W2, b2, W3, b3):
    """Host epilogue: piece sums -> segment sums -> mean -> W3."""
    B = plan["B"]
    counts = plan["counts"].astype(np.float32)
    # zero-pad cols produce relu(0 @ W1 + b1*s) = s*relu(b1): subtract
    relu_b1 = np.maximum(np.asarray(b1, np.float32), 0.0)

    p8, p16 = plan["p8"], plan["p16"]
    nA = p8["nA"] + p16["nA"]
    # acc layout: [ACT8 | ACT16 | DVE8 | DVE16]
    eng_of = {}
    for cls, plan_c in (("p8", p8), ("p16", p16)):
        eng_of[cls] = {slot: e_ for (t, off, fd, e_, slot) in plan_c["sched"]}

    def col_global(cls, s):
        plan_c = plan[cls]
        e_ = eng_of[cls][s]
        col = plan_c["col_of"][s]
        if e_ == 0:
            return col + (0 if cls == "p8" else p8["nA"])
        return nA + col + (0 if cls == "p8" else p8["nD"])

    R = np.zeros((B, 128), dtype=np.float64)
    for c in range(N_CORES):
        acc = np.asarray(acc_list[c], np.float64)
        for cls, npads, scale in (("p8", plan["npads8"][c], SCALE8),
                                  ("p16", plan["npads16"][c], 1.0)):
            plan_c = plan[cls]
            for s, piece in enumerate(plan_c["core_pieces"][c]):
                if piece is None:
                    continue
                seg = piece[0]
                R[seg] += acc[:, col_global(cls, s)] / scale
                if npads[s]:
                    R[seg] -= npads[s] * relu_b1
    R = R.astype(np.float32)

    W2 = np.asarray(W2, np.float32)
    b2 = np.asarray(b2, np.float32)
    W3 = np.asarray(W3, np.float32)
    b3 = np.asarray(b3, np.float32)
    sums_h = R @ W2 + counts[:, None] * b2[None, :]
    mean = sums_h / np.maximum(counts, 1.0)[:, None]
    return (mean @ W3 + b3[None, :]).astype(np.float32)


def kernel(x1, edge_slices, W1, b1, W2, b2, W3, b3):
    from concourse import bass_utils

    nc, in_maps, plan = _prepare(x1, edge_slices, W1, b1, W2, b2, W3, b3)
    br = bass_utils.run_bass_kernel_spmd(
        nc, in_maps, core_ids=list(range(N_CORES)))
    return _finish([r["acc"] for r in br.results], plan, b1, W2, b2, W3, b3)
